# Initial kernel scaffold
#
"""Causal self-attention with YaRN RoPE on 8 TRN2 NeuronCores.

Sharding: data-parallel over batch (B=2) x tensor-parallel over head
groups (16 heads -> 4 groups of 4). Core c handles batch c//4, heads
4*(c%4)..4*(c%4)+3. Each core computes its partial c_proj output
[T, C]; the host sums the 4 partials per batch (the "all-reduce").

Device layout: everything "transposed" (channels on partitions, time on
free dim) so no on-device transposes are needed:
  qT/kT:  out[o, t] = sum_c Wqk[c, o] * xT[c, t]   (lhsT=W native, rhs=xT)
  V:      out[t, o] = sum_c xT[c, t] * Wv[c, o]    (lhsT=xT, rhs=W native)
  RoPE:   rotate_half done as a 128x128 constant matmul (R2T) + 3 DVE TTs
  Sᵀ:     scoresT[k, q] = sum_d kT[d,k] * qT[d,q], 2 heads row-packed
          (K=64 each at array rows 0-63 / 64-127, concurrent MMs)
  softmax: exp on ACT (scale=1/8 fused), no max subtraction (|s|<~8),
          causal via AV N-trimming + tril mask-mul on diagonal blocks,
          denominator via ones-column appended to V (M=65 AV matmuls)
  AV:     yT[d, q] = sum_k V[k, d] * probsT[k, q]  (lhsT=V_ext, rhs=probsT)
  norm:   reciprocal(denoms) once, broadcast via K=2 matmul, one TT mul
  proj:   out[t, n] = sum_o yT[o, t] * Wp[o, n]    (lhsT=yT native)
"""

import math

import numpy as np

D_MODEL = 1024
N_HEAD = 16
D_HEAD = 64
ORIG_MAX_SEQ = 1024
BASE = 10000.0
B, T, C = 2, 2048, 1024
N_CORES = 8


def _rope_cos_sin(Tn):
    """Mirror of reference.rope_cos_sin in numpy (float64 then cast)."""
    dim = D_HEAD
    inv_freq = 1.0 / BASE ** (np.arange(0, dim, 2, dtype=np.float64) / dim)
    scale = max(1.0, Tn / ORIG_MAX_SEQ)
    t = np.arange(Tn, dtype=np.float64)
    if scale <= 1.0:
        freqs = t[:, None] * inv_freq
        emb = np.concatenate((freqs, freqs), axis=-1)
        return np.cos(emb).astype(np.float32), np.sin(emb).astype(np.float32)
    new_base = BASE * scale ** (dim / (dim - 2))
    inv_freq_yarn = 1.0 / new_base ** (np.arange(0, dim, 2, dtype=np.float64) / dim)
    beta_fast, beta_slow = 32.0, 1.0
    num_rot = np.arange(0, dim, 2, dtype=np.float64) / dim
    ramp = np.clip((num_rot - beta_slow) / (beta_fast - beta_slow), 0.0, 1.0)
    inv_freq_yarn = inv_freq_yarn * (1 - ramp) + (inv_freq / scale) * ramp
    mscale = 0.1 * math.log(scale) + 1.0
    freqs = t[:, None] * inv_freq_yarn
    emb = np.concatenate((freqs, freqs), axis=-1)
    return (np.cos(emb) * mscale).astype(np.float32), (
        np.sin(emb) * mscale
    ).astype(np.float32)


def _emit(tc, d, mybir, nloop=1):
    import concourse.bass as bass

    nc = tc.nc
    f32 = mybir.dt.float32
    PSUM = bass.MemorySpace.PSUM
    Exp = mybir.ActivationFunctionType.Exp
    mult = mybir.AluOpType.mult
    add = mybir.AluOpType.add

    from contextlib import ExitStack

    ctx = tc._kernel_ctx  # ExitStack we stash on tc before calling

    # ---- persistent pools -------------------------------------------------
    consts = ctx.enter_context(tc.tile_pool(name="consts", bufs=1))
    ropep = ctx.enter_context(tc.tile_pool(name="rope", bufs=1))
    vextp = ctx.enter_context(tc.tile_pool(name="vext", bufs=1))
    ytp = ctx.enter_context(tc.tile_pool(name="yt", bufs=1))
    stagep = ctx.enter_context(tc.tile_pool(name="stage", bufs=1))

    cosT = consts.tile([128, T], f32, tag="cosT")
    nc.sync.dma_start(cosT[:], d["cosT"][:])
    sinT = consts.tile([128, T], f32, tag="sinT")
    nc.sync.dma_start(sinT[:], d["sinT"][:])
    r2t = consts.tile([128, 128], f32, tag="r2t")
    nc.sync.dma_start(r2t[:], d["r2t"][:])
    tril = consts.tile([128, 128], f32, tag="tril")
    nc.sync.dma_start(tril[:], d["tril"][:])
    e2 = consts.tile([2, 128], f32, tag="e2")
    nc.sync.dma_start(e2[:], d["e2"][:])
    wp = []
    for p in range(2):
        w_ = consts.tile([128, C], f32, tag=f"wp{p}")
        nc.sync.dma_start(w_[:], d["wp"][128 * p : 128 * (p + 1), :])
        wp.append(w_)

    rope_q = [ropep.tile([128, T], f32, tag=f"rq{p}") for p in range(2)]
    rope_k = [ropep.tile([128, T], f32, tag=f"rk{p}") for p in range(2)]
    vext = [vextp.tile([128, 260], f32, tag=f"vx{i}") for i in range(16)]
    ytn = [ytp.tile([128, T], f32, tag=f"ytn{p}") for p in range(2)]
    stage = stagep.tile([16, 512], f32, tag="stage")
    rstage = stagep.tile([16, 512], f32, tag="rstage")

    # ones columns of V_ext (col 64 of each head's 65-wide strip)
    for i in range(16):
        for h in range(4):
            nc.gpsimd.memset(vext[i][:, 65 * h + 64 : 65 * h + 65], 1.0)

    # ---- phase 1: QKV + RoPE ---------------------------------------------
    with (
        tc.tile_pool(name="xt", bufs=1) as xtp,
        tc.tile_pool(name="wgp", bufs=1) as wgp,
        tc.tile_pool(name="rawqk", bufs=2) as rawp,
        tc.tile_pool(name="qkps", bufs=4, space=PSUM) as qkps_pool,
        tc.tile_pool(name="rotps", bufs=2, space=PSUM) as rotps_pool,
        tc.tile_pool(name="vps", bufs=2, space=PSUM) as vps_pool,
    ):
        xt = []
        for cc in range(8):
            t_ = xtp.tile([128, T], f32, tag=f"xt{cc}")
            nc.sync.dma_start(t_[:], d["xT"][128 * cc : 128 * (cc + 1), :])
            xt.append(t_)
        wg = []
        for cc in range(8):
            t_ = wgp.tile([128, 768], f32, tag=f"wg{cc}")
            nc.sync.dma_start(t_[:], d["wg"][128 * cc : 128 * (cc + 1), :])
            wg.append(t_)

        # q/k projections (transposed layout) + rope
        for oc in range(4):  # oc 0,1 = q pairs; 2,3 = k pairs
            qk_ps = [qkps_pool.tile([128, 512], f32, tag="qkps") for _ in range(4)]
            for cc in range(8):
                for t4 in range(4):
                    nc.tensor.matmul(
                        qk_ps[t4][:],
                        wg[cc][:, 128 * oc : 128 * (oc + 1)],
                        xt[cc][:, 512 * t4 : 512 * (t4 + 1)],
                        start=(cc == 0),
                        stop=(cc == 7),
                    )
            raw = rawp.tile([128, T], f32, tag="raw")
            dest = rope_q[oc] if oc < 2 else rope_k[oc - 2]
            for t4 in range(4):
                nc.vector.tensor_copy(raw[:, 512 * t4 : 512 * (t4 + 1)], qk_ps[t4][:])
            for t4 in range(4):
                sl = slice(512 * t4, 512 * (t4 + 1))
                rp = rotps_pool.tile([128, 512], f32, tag="rotps")
                nc.tensor.matmul(rp[:], r2t[:], raw[:, sl], start=True, stop=True)
                nc.vector.tensor_tensor(dest[:, sl], qk_ps[t4][:], cosT[:, sl], op=mult)
                tmp = rawp.tile([128, 512], f32, tag="ropetmp")
                nc.vector.tensor_tensor(tmp[:], rp[:], sinT[:, sl], op=mult)
                nc.vector.tensor_tensor(dest[:, sl], dest[:, sl], tmp[:], op=add)

        # V projection (normal layout) into V_ext strips
        for t16 in range(16):
            v_ps = vps_pool.tile([128, 256], f32, tag="vps")
            for cc in range(8):
                nc.tensor.matmul(
                    v_ps[:],
                    xt[cc][:, 128 * t16 : 128 * (t16 + 1)],
                    wg[cc][:, 512:768],
                    start=(cc == 0),
                    stop=(cc == 7),
                )
            vo = vext[t16][:].rearrange("p (h c) -> p h c", c=65)[:, :, 0:64]
            vi = v_ps[:].rearrange("p (h c) -> p h c", c=64)
            nc.vector.tensor_copy(vo, vi)

    # ---- phase 2: attention ----------------------------------------------
    with (
        tc.tile_pool(name="probs", bufs=4) as probsp,
        tc.tile_pool(name="sps", bufs=3, space=PSUM) as sps_pool,
        tc.tile_pool(name="ytps", bufs=2, space=PSUM) as ytps_pool,
    ):
        for p in range(2):
            rq, rk = rope_q[p], rope_k[p]
            for j in range(4):
                n_i = 4 * j + 4
                LOOKAHEAD = 2
                probs_tiles = {}
                yt_ps = [ytps_pool.tile([65, 512], f32, tag="ytps") for _ in range(2)]

                def scores(i):
                    S = sps_pool.tile([128, 1024], f32, tag="sps")
                    for a in range(2):
                        nc.tensor.matmul(
                            S[:, 512 * a : 512 * (a + 1)],
                            rk[64 * a : 64 * (a + 1), 128 * i : 128 * (i + 1)],
                            rq[64 * a : 64 * (a + 1), 512 * j : 512 * (j + 1)],
                            start=True,
                            stop=True,
                        )
                    P = probsp.tile([128, 1024], f32, tag="probs")
                    nc.scalar.activation(P[:], S[:], Exp, scale=0.125)
                    r = i - 4 * j
                    if r >= 0:
                        for a in range(2):
                            sl = P[:, 512 * a + 128 * r : 512 * a + 128 * r + 128]
                            nc.vector.tensor_tensor(sl, sl, tril[:], op=mult)
                    probs_tiles[i] = P

                def av(i):
                    P = probs_tiles.pop(i)
                    r = i - 4 * j
                    lo = 128 * r if r > 0 else 0
                    for a in range(2):
                        h = 2 * p + a
                        nc.tensor.matmul(
                            yt_ps[a][:, lo:512],
                            vext[i][:, 65 * h : 65 * h + 65],
                            P[:, 512 * a + lo : 512 * a + 512],
                            start=(i == 0),
                            stop=(i == n_i - 1),
                            skip_group_check=True,
                        )

                for idx in range(n_i + LOOKAHEAD):
                    if idx < n_i:
                        scores(idx)
                    if idx >= LOOKAHEAD:
                        av(idx - LOOKAHEAD)

                row = 2 * (p * 4 + j)
                for a in range(2):
                    nc.vector.tensor_copy(
                        ytn[p][64 * a : 64 * (a + 1), 512 * j : 512 * (j + 1)],
                        yt_ps[a][0:64, :],
                    )
                    nc.vector.tensor_copy(
                        stage[row + a : row + a + 1, :], yt_ps[a][64:65, :]
                    )

    # ---- phase 3: normalize + proj + out ----------------------------------
    with (
        tc.tile_pool(name="bc", bufs=2, space=PSUM) as bcp,
        tc.tile_pool(name="outp", bufs=3) as outp,
        tc.tile_pool(name="pps", bufs=4, space=PSUM) as pps,
    ):
        nc.vector.reciprocal(rstage[:], stage[:])
        for p in range(2):
            for j in range(4):
                row = 2 * (p * 4 + j)
                b_ps = bcp.tile([128, 512], f32, tag="bc")
                nc.tensor.matmul(
                    b_ps[:], e2[:], rstage[row : row + 2, :], start=True, stop=True
                )
                sl = ytn[p][:, 512 * j : 512 * (j + 1)]
                nc.vector.tensor_tensor(sl, sl, b_ps[:], op=mult)
        for t16 in range(16):
            for n2 in range(2):
                o_ps = pps.tile([128, 512], f32, tag="pps")
                for p in range(2):
                    nc.tensor.matmul(
                        o_ps[:],
                        ytn[p][:, 128 * t16 : 128 * (t16 + 1)],
                        wp[p][:, 512 * n2 : 512 * (n2 + 1)],
                        start=(p == 0),
                        stop=(p == 1),
                    )
                ob = outp.tile([128, 512], f32, tag="ob")
                nc.vector.tensor_copy(ob[:], o_ps[:])
                nc.sync.dma_start(
                    d["out"][128 * t16 : 128 * (t16 + 1), 512 * n2 : 512 * (n2 + 1)],
                    ob[:],
                )


_PROGRAM_CACHE = {}


def build_program(nloop=1):
    """Build + compile the single-core SPMD program. Cached per process."""
    if nloop in _PROGRAM_CACHE:
        return _PROGRAM_CACHE[nloop]
    from contextlib import ExitStack

    import concourse.tile as tile
    from concourse import bacc, mybir

    f32 = mybir.dt.float32
    nc = bacc.Bacc("TRN2", target_bir_lowering=False, debug=False, num_devices=N_CORES)
    d = {}
    d["xT"] = nc.declare_dram_parameter("xT", [C, T], f32, isOutput=False)
    d["wg"] = nc.declare_dram_parameter("wg", [C, 768], f32, isOutput=False)
    d["wp"] = nc.declare_dram_parameter("wp", [256, C], f32, isOutput=False)
    d["cosT"] = nc.declare_dram_parameter("cosT", [128, T], f32, isOutput=False)
    d["sinT"] = nc.declare_dram_parameter("sinT", [128, T], f32, isOutput=False)
    d["r2t"] = nc.declare_dram_parameter("r2t", [128, 128], f32, isOutput=False)
    d["tril"] = nc.declare_dram_parameter("tril", [128, 128], f32, isOutput=False)
    d["e2"] = nc.declare_dram_parameter("e2", [2, 128], f32, isOutput=False)
    d["out"] = nc.declare_dram_parameter("out", [T, C], f32, isOutput=True)

    with ExitStack() as ctx:
        with tile.TileContext(nc) as tc:
            tc._kernel_ctx = ctx
            _emit(tc, d, mybir, nloop=nloop)
    nc.compile()
    _PROGRAM_CACHE[nloop] = nc
    return nc


def make_in_maps(x, W_attn, W_proj):
    """Shard full inputs into per-core input maps (host side)."""
    x = np.asarray(x, dtype=np.float32)
    W_attn = np.asarray(W_attn, dtype=np.float32)
    W_proj = np.asarray(W_proj, dtype=np.float32)

    cos, sin = _rope_cos_sin(T)  # [T, 64]
    cosT = np.ascontiguousarray(np.concatenate([cos.T, cos.T], axis=0))  # [128,T]
    sinT = np.ascontiguousarray(np.concatenate([sin.T, sin.T], axis=0))

    # rotate_half as matrix: rot(q)[dd] = -q[dd+32] (dd<32), q[dd-32] (dd>=32)
    R = np.zeros((64, 64), dtype=np.float32)
    for dd in range(32):
        R[dd, dd + 32] = -1.0
        R[dd + 32, dd] = 1.0
    R2 = np.zeros((128, 128), dtype=np.float32)
    R2[0:64, 0:64] = R
    R2[64:128, 64:128] = R
    r2t = np.ascontiguousarray(R2.T)  # lhsT[d', d] = R2[d, d']

    # scoresT[k, q] valid where k <= q (within the diagonal 128-block)
    trilm = np.triu(np.ones((128, 128), dtype=np.float32))

    e2 = np.zeros((2, 128), dtype=np.float32)
    e2[0, 0:64] = 1.0
    e2[1, 64:128] = 1.0

    in_maps = []
    for c in range(N_CORES):
        b, g = c // 4, c % 4
        xT = np.ascontiguousarray(x[b].T)  # [C, T]
        wg = np.ascontiguousarray(
            np.concatenate(
                [
                    W_attn[:, 256 * g : 256 * (g + 1)],
                    W_attn[:, 1024 + 256 * g : 1024 + 256 * (g + 1)],
                    W_attn[:, 2048 + 256 * g : 2048 + 256 * (g + 1)],
                ],
                axis=1,
            )
        )
        wpg = np.ascontiguousarray(W_proj[256 * g : 256 * (g + 1), :])
        in_maps.append(
            {
                "xT": xT,
                "wg": wg,
                "wp": wpg,
                "cosT": cosT,
                "sinT": sinT,
                "r2t": r2t,
                "tril": trilm,
                "e2": e2,
            }
        )
    return in_maps


def kernel(x, W_attn, W_proj):
    from concourse.bass_utils import run_bass_kernel_spmd

    nc = build_program()
    in_maps = make_in_maps(x, W_attn, W_proj)
    res = run_bass_kernel_spmd(nc, in_maps, list(range(N_CORES)), trace=False).results
    out = np.zeros((B, T, C), dtype=np.float32)
    for c in range(N_CORES):
        out[c // 4] += res[c]["out"]
    return out


# revision 14
# speedup vs baseline: 7172.4686x; 7172.4686x over previous
"""Causal self-attention with YaRN RoPE on 8 TRN2 NeuronCores.

Sharding: data-parallel over batch (B=2) x tensor-parallel over head
groups (16 heads -> 4 groups of 4). Core c handles batch c//4, heads
4*(c%4)..4*(c%4)+3. Each core computes its partial c_proj output
[T, C]; the host sums the 4 partials per batch (the "all-reduce").

Device layout: everything "transposed" (channels on partitions, time on
free dim) so no on-device transposes are needed:
  qT/kT:  out[o, t] = sum_c Wqk[c, o] * xT[c, t]   (lhsT=W native, rhs=xT)
  V:      out[t, o] = sum_c xT[c, t] * Wv[c, o]    (lhsT=xT, rhs=W native)
  RoPE:   rotate_half done as a 128x128 constant matmul (R2T) + 3 DVE TTs
  Sᵀ:     scoresT[k, q] = sum_d kT[d,k] * qT[d,q], 2 heads row-packed
          (K=64 each at array rows 0-63 / 64-127, concurrent MMs)
  softmax: exp on ACT (scale=1/8 fused), no max subtraction (|s|<~8),
          causal via AV N-trimming + tril mask-mul on diagonal blocks,
          denominator via ones-column appended to V (M=65 AV matmuls)
  AV:     yT[d, q] = sum_k V[k, d] * probsT[k, q]  (lhsT=V_ext, rhs=probsT)
  norm:   reciprocal(denoms) once, broadcast via K=2 matmul, one TT mul
  proj:   out[t, n] = sum_o yT[o, t] * Wp[o, n]    (lhsT=yT native)
"""

import math

import numpy as np

D_MODEL = 1024
N_HEAD = 16
D_HEAD = 64
ORIG_MAX_SEQ = 1024
BASE = 10000.0
B, T, C = 2, 2048, 1024
N_CORES = 8


def _rope_cos_sin(Tn):
    """Mirror of reference.rope_cos_sin in numpy (float64 then cast)."""
    dim = D_HEAD
    inv_freq = 1.0 / BASE ** (np.arange(0, dim, 2, dtype=np.float64) / dim)
    scale = max(1.0, Tn / ORIG_MAX_SEQ)
    t = np.arange(Tn, dtype=np.float64)
    if scale <= 1.0:
        freqs = t[:, None] * inv_freq
        emb = np.concatenate((freqs, freqs), axis=-1)
        return np.cos(emb).astype(np.float32), np.sin(emb).astype(np.float32)
    new_base = BASE * scale ** (dim / (dim - 2))
    inv_freq_yarn = 1.0 / new_base ** (np.arange(0, dim, 2, dtype=np.float64) / dim)
    beta_fast, beta_slow = 32.0, 1.0
    num_rot = np.arange(0, dim, 2, dtype=np.float64) / dim
    ramp = np.clip((num_rot - beta_slow) / (beta_fast - beta_slow), 0.0, 1.0)
    inv_freq_yarn = inv_freq_yarn * (1 - ramp) + (inv_freq / scale) * ramp
    mscale = 0.1 * math.log(scale) + 1.0
    freqs = t[:, None] * inv_freq_yarn
    emb = np.concatenate((freqs, freqs), axis=-1)
    return (np.cos(emb) * mscale).astype(np.float32), (
        np.sin(emb) * mscale
    ).astype(np.float32)


def _emit(tc, d, mybir, nloop=1):
    import contextlib

    import concourse.bass as bass

    nc = tc.nc
    f32 = mybir.dt.float32
    bf16 = mybir.dt.bfloat16
    PSUM = bass.MemorySpace.PSUM
    Exp = mybir.ActivationFunctionType.Exp
    Ln = mybir.ActivationFunctionType.Ln
    mult = mybir.AluOpType.mult
    add = mybir.AluOpType.add

    ctx = tc._kernel_ctx  # ExitStack closed before TileContext exit

    # ---- pools (all flat so the body can sit inside a For_i loop) ---------
    consts = ctx.enter_context(tc.tile_pool(name="consts", bufs=1))
    ropep = ctx.enter_context(tc.tile_pool(name="rope", bufs=1))
    vextp = ctx.enter_context(tc.tile_pool(name="vext", bufs=1))
    ytp = ctx.enter_context(tc.tile_pool(name="yt", bufs=1))
    stagep = ctx.enter_context(tc.tile_pool(name="stage", bufs=1))
    xtp = ctx.enter_context(tc.tile_pool(name="xt", bufs=1))
    wgp = ctx.enter_context(tc.tile_pool(name="wgp", bufs=1))
    rawp = ctx.enter_context(tc.tile_pool(name="rawqk", bufs=2))
    probsp = ctx.enter_context(tc.tile_pool(name="probs", bufs=3))
    outp = ctx.enter_context(tc.tile_pool(name="outp", bufs=2))
    # PSUM: "big" [128,1024] x3 (6 banks) + "small" [128,512] x2 (2 banks)
    psA = ctx.enter_context(tc.tile_pool(name="psA", bufs=3, space=PSUM))
    psB = ctx.enter_context(tc.tile_pool(name="psB", bufs=2, space=PSUM))

    loop_cm = (
        tc.For_i(0, nloop, 1, hint_engines=(mybir.EngineType.PE,))
        if nloop > 1
        else contextlib.nullcontext()
    )
    with loop_cm:
        cosT = consts.tile([128, T], f32, tag="cosT")
        nc.sync.dma_start(cosT[:], d["cosT"][:])
        sinT = consts.tile([128, T], f32, tag="sinT")
        nc.sync.dma_start(sinT[:], d["sinT"][:])
        r2t = consts.tile([128, 128], f32, tag="r2t")
        nc.sync.dma_start(r2t[:], d["r2t"][:])
        tril = consts.tile([128, 128], f32, tag="tril")
        nc.sync.dma_start(tril[:], d["tril"][:])
        wp = []
        for p in range(2):
            w_ = consts.tile([128, C], f32, tag=f"wp{p}")
            nc.sync.dma_start(w_[:], d["wp"][128 * p : 128 * (p + 1), :])
            wp.append(w_)

        rope_q = [
            ropep.tile([128, T], f32, tag=f"rq{p}", name=f"rq{p}") for p in range(2)
        ]
        rope_k = [
            ropep.tile([128, T], f32, tag=f"rk{p}", name=f"rk{p}") for p in range(2)
        ]
        vext = [
            vextp.tile([128, 260], bf16, tag=f"vx{i}", name=f"vx{i}")
            for i in range(16)
        ]
        trilb = consts.tile([128, 128], bf16, tag="trilb")
        nc.vector.tensor_copy(trilb[:], tril[:])
        ytn = [ytp.tile([128, T], f32, tag=f"ytn{p}", name=f"ytn{p}") for p in range(2)]
        # Engine ops need start partitions in {0,32,64,96}, so denominator
        # row (p,j,a) -> idx = 2*(4p+j)+a lives at partition 32*(idx%4) of
        # stageA[idx//4] ([1,512] strips at legal bases only).
        stageA = [
            stagep.tile([128, 512], f32, tag=f"stA{i}", name=f"stA{i}")
            for i in range(4)
        ]
        stageB = stagep.tile([128, 512], f32, tag="stB")
        for i in range(4):
            nc.vector.memset(stageA[i][:], 1.0)  # keep ln() NaN-free off-rows
        ones4 = stagep.tile([128, 64], f32, tag="ones4")
        nc.vector.memset(ones4[:], 1.0)

        # ones columns of V_ext (col 64 of each head's 65-wide strip)
        for i in range(16):
            for h in range(4):
                nc.gpsimd.memset(vext[i][:, 65 * h + 64 : 65 * h + 65], 1.0)

        # ---- phase 1: QKV + RoPE -----------------------------------------
        xt = []
        for cc in range(8):
            t_ = xtp.tile([128, T], f32, tag=f"xt{cc}", name=f"xt{cc}")
            nc.sync.dma_start(t_[:], d["xT"][128 * cc : 128 * (cc + 1), :])
            xt.append(t_)
        wg = []
        for cc in range(8):
            t_ = wgp.tile([128, 768], f32, tag=f"wg{cc}", name=f"wg{cc}")
            nc.sync.dma_start(t_[:], d["wg"][128 * cc : 128 * (cc + 1), :])
            wg.append(t_)

        # q/k projections (transposed layout) + rope
        for oc in range(4):  # oc 0,1 = q pairs; 2,3 = k pairs
            qk_ps = [
                psA.tile([128, 1024], f32, tag="big", name="qk_ps") for _ in range(2)
            ]
            for cc in range(8):
                for t4 in range(4):
                    nc.tensor.matmul(
                        qk_ps[t4 // 2][:, 512 * (t4 % 2) : 512 * (t4 % 2 + 1)],
                        wg[cc][:, 128 * oc : 128 * (oc + 1)],
                        xt[cc][:, 512 * t4 : 512 * (t4 + 1)],
                        start=(cc == 0),
                        stop=(cc == 7),
                    )
            dest = rope_q[oc] if oc < 2 else rope_k[oc - 2]
            for t4 in range(4):
                sl = slice(512 * t4, 512 * (t4 + 1))
                qsl = qk_ps[t4 // 2][:, 512 * (t4 % 2) : 512 * (t4 % 2 + 1)]
                raw = rawp.tile([128, 512], f32, tag="raw", bufs=1)
                nc.vector.tensor_copy(raw[:], qsl)
                rp = psB.tile([128, 512], f32, tag="small", name="rot_ps")
                nc.tensor.matmul(rp[:], r2t[:], raw[:], start=True, stop=True)
                nc.vector.tensor_tensor(dest[:, sl], qsl, cosT[:, sl], op=mult)
                tmp = rawp.tile([128, 512], f32, tag="ropetmp")
                nc.vector.tensor_tensor(tmp[:], rp[:], sinT[:, sl], op=mult)
                nc.vector.tensor_tensor(dest[:, sl], dest[:, sl], tmp[:], op=add)

        # V projection (normal layout) into V_ext strips
        for t16 in range(16):
            v_ps = psB.tile([128, 512], f32, tag="small", name="v_ps")
            for cc in range(8):
                nc.tensor.matmul(
                    v_ps[:, 0:256],
                    xt[cc][:, 128 * t16 : 128 * (t16 + 1)],
                    wg[cc][:, 512:768],
                    start=(cc == 0),
                    stop=(cc == 7),
                )
            vo = vext[t16][:].rearrange("p (h c) -> p h c", c=65)[:, :, 0:64]
            vi = v_ps[:, 0:256].rearrange("p (h c) -> p h c", c=64)
            nc.vector.tensor_copy(vo, vi)

        # ---- phase 2: attention ------------------------------------------
        for p in range(2):
            rq, rk = rope_q[p], rope_k[p]
            for j in range(4):
                n_i = 4 * j + 4
                LOOKAHEAD = 2
                probs_tiles = {}
                yt_ps = [
                    psB.tile([65, 512], f32, tag="small", name="yt_ps")
                    for _ in range(2)
                ]

                def scores(i):
                    S = psA.tile([128, 1024], f32, tag="big", name="S")
                    for a in range(2):
                        nc.tensor.matmul(
                            S[:, 512 * a : 512 * (a + 1)],
                            rk[64 * a : 64 * (a + 1), 128 * i : 128 * (i + 1)],
                            rq[64 * a : 64 * (a + 1), 512 * j : 512 * (j + 1)],
                            start=True,
                            stop=True,
                        )
                    P = probsp.tile([128, 1024], bf16, tag="probs")
                    nc.scalar.activation(P[:], S[:], Exp, scale=0.125)
                    r = i - 4 * j
                    if r >= 0:
                        for a in range(2):
                            sl = P[:, 512 * a + 128 * r : 512 * a + 128 * (r + 1)]
                            nc.vector.tensor_tensor(sl, sl, trilb[:], op=mult)
                    probs_tiles[i] = P

                def av(i):
                    P = probs_tiles.pop(i)
                    r = i - 4 * j
                    lo = 128 * r if r > 0 else 0
                    for a in range(2):
                        h = 2 * p + a
                        nc.tensor.matmul(
                            yt_ps[a][:, lo:512],
                            vext[i][:, 65 * h : 65 * h + 65],
                            P[:, 512 * a + lo : 512 * a + 512],
                            start=(i == 0),
                            stop=(i == n_i - 1),
                            skip_group_check=True,
                        )

                for idx in range(n_i + LOOKAHEAD):
                    if idx < n_i:
                        scores(idx)
                    if idx >= LOOKAHEAD:
                        av(idx - LOOKAHEAD)

                row = 2 * (p * 4 + j)
                for a in range(2):
                    nc.vector.tensor_copy(
                        ytn[p][64 * a : 64 * (a + 1), 512 * j : 512 * (j + 1)],
                        yt_ps[a][0:64, :],
                    )
                    idx = row + a
                    pp = 32 * (idx % 4)
                    nc.vector.tensor_copy(
                        stageA[idx // 4][pp : pp + 1, :], yt_ps[a][64:65, :]
                    )

        # ---- phase 3: normalize + proj + out ------------------------------
        # 1/denom via exp(-ln(x)) on ACT (vector.reciprocal is 8 cyc/elem;
        # ACT streams at 1/cyc and Ln/Exp splines are ~2 ULP).
        for i in range(4):
            nc.scalar.activation(stageB[:], stageA[i][:], Ln)
            nc.scalar.activation(stageA[i][:], stageB[:], Exp, scale=-1.0)
        # Broadcast each [1,512] recip row across partitions with K=1 matmuls.
        for p in range(2):
            for j in range(4):
                row = 2 * (p * 4 + j)
                b_ps = psB.tile([128, 512], f32, tag="small", name="b_ps")
                for a in range(2):
                    idx = row + a
                    pp = 32 * (idx % 4)
                    nc.tensor.matmul(
                        b_ps[64 * a : 64 * (a + 1), :],
                        ones4[pp : pp + 1, :],
                        stageA[idx // 4][pp : pp + 1, :],
                        start=True,
                        stop=True,
                        tile_position=(pp, 64 * a),
                    )
                sl = ytn[p][:, 512 * j : 512 * (j + 1)]
                nc.vector.tensor_tensor(sl, sl, b_ps[:], op=mult)
        for t16 in range(16):
            for n2 in range(2):
                o_ps = psB.tile([128, 512], f32, tag="small", name="o_ps")
                for p in range(2):
                    nc.tensor.matmul(
                        o_ps[:],
                        ytn[p][:, 128 * t16 : 128 * (t16 + 1)],
                        wp[p][:, 512 * n2 : 512 * (n2 + 1)],
                        start=(p == 0),
                        stop=(p == 1),
                    )
                ob = outp.tile([128, 512], f32, tag="ob")
                nc.vector.tensor_copy(ob[:], o_ps[:])
                nc.sync.dma_start(
                    d["out"][128 * t16 : 128 * (t16 + 1), 512 * n2 : 512 * (n2 + 1)],
                    ob[:],
                )


_PROGRAM_CACHE = {}


def build_program(nloop=1):
    """Build + compile the single-core SPMD program. Cached per process."""
    if nloop in _PROGRAM_CACHE:
        return _PROGRAM_CACHE[nloop]
    from contextlib import ExitStack

    import concourse.tile as tile
    from concourse import bacc, mybir

    f32 = mybir.dt.float32
    nc = bacc.Bacc("TRN2", target_bir_lowering=False, debug=False, num_devices=N_CORES)
    d = {}
    d["xT"] = nc.declare_dram_parameter("xT", [C, T], f32, isOutput=False)
    d["wg"] = nc.declare_dram_parameter("wg", [C, 768], f32, isOutput=False)
    d["wp"] = nc.declare_dram_parameter("wp", [256, C], f32, isOutput=False)
    d["cosT"] = nc.declare_dram_parameter("cosT", [128, T], f32, isOutput=False)
    d["sinT"] = nc.declare_dram_parameter("sinT", [128, T], f32, isOutput=False)
    d["r2t"] = nc.declare_dram_parameter("r2t", [128, 128], f32, isOutput=False)
    d["tril"] = nc.declare_dram_parameter("tril", [128, 128], f32, isOutput=False)
    d["out"] = nc.declare_dram_parameter("out", [T, C], f32, isOutput=True)

    with tile.TileContext(nc) as tc:
        with ExitStack() as ctx:
            tc._kernel_ctx = ctx
            _emit(tc, d, mybir, nloop=nloop)
    nc.compile()
    _PROGRAM_CACHE[nloop] = nc
    return nc


def make_in_maps(x, W_attn, W_proj):
    """Shard full inputs into per-core input maps (host side)."""
    x = np.asarray(x, dtype=np.float32)
    W_attn = np.asarray(W_attn, dtype=np.float32)
    W_proj = np.asarray(W_proj, dtype=np.float32)

    cos, sin = _rope_cos_sin(T)  # [T, 64]
    cosT = np.ascontiguousarray(np.concatenate([cos.T, cos.T], axis=0))  # [128,T]
    sinT = np.ascontiguousarray(np.concatenate([sin.T, sin.T], axis=0))

    # rotate_half as matrix: rot(q)[dd] = -q[dd+32] (dd<32), q[dd-32] (dd>=32)
    R = np.zeros((64, 64), dtype=np.float32)
    for dd in range(32):
        R[dd, dd + 32] = -1.0
        R[dd + 32, dd] = 1.0
    R2 = np.zeros((128, 128), dtype=np.float32)
    R2[0:64, 0:64] = R
    R2[64:128, 64:128] = R
    r2t = np.ascontiguousarray(R2.T)  # lhsT[d', d] = R2[d, d']

    # scoresT[k, q] valid where k <= q (within the diagonal 128-block)
    trilm = np.triu(np.ones((128, 128), dtype=np.float32))

    in_maps = []
    for c in range(N_CORES):
        b, g = c // 4, c % 4
        xT = np.ascontiguousarray(x[b].T)  # [C, T]
        wg = np.ascontiguousarray(
            np.concatenate(
                [
                    W_attn[:, 256 * g : 256 * (g + 1)],
                    W_attn[:, 1024 + 256 * g : 1024 + 256 * (g + 1)],
                    W_attn[:, 2048 + 256 * g : 2048 + 256 * (g + 1)],
                ],
                axis=1,
            )
        )
        wpg = np.ascontiguousarray(W_proj[256 * g : 256 * (g + 1), :])
        in_maps.append(
            {
                "xT": xT,
                "wg": wg,
                "wp": wpg,
                "cosT": cosT,
                "sinT": sinT,
                "r2t": r2t,
                "tril": trilm,
            }
        )
    return in_maps


def kernel(x, W_attn, W_proj):
    from concourse.bass_utils import run_bass_kernel_spmd

    nc = build_program()
    in_maps = make_in_maps(x, W_attn, W_proj)
    res = run_bass_kernel_spmd(nc, in_maps, list(range(N_CORES)), trace=False).results
    out = np.zeros((B, T, C), dtype=np.float32)
    for c in range(N_CORES):
        out[c // 4] += res[c]["out"]
    return out
